# revision 2
# baseline (speedup 1.0000x reference)
"""V4: mega-gather, mega-chunk design.

Math: s(edge) = g[e0] + g[e1] + b_edge, g = relu(emb@W+b) @ wbar,
wbar = (W_edge[:64] + W_edge[64:]) / 2.  out = sigmoid(logit(eps) + s).

Device work per core (25088 edge-slots per partition-group, 8 groups):
  phase 1: g for own 6250 nodes -> AllGather g (200KB) -> select-16
    table [128, 3125] (table[16G+c, w] = g[16w+c], replicated over G).
  phase 2 gathers (POOL, 8 instructions total):
    - src: edges host-sorted by e0, grouped 8-per-bucket (w=e0>>4);
      ONE indirect_copy (3392 cols); each gathered col serves 8 slots
      per group via a step-0 broadcast AP in the select.
    - dst: 7 chunked indirect_copy calls (27136 cols total, 1 col = 8
      slots, one per partition-group).
  select-16 per 2048-col chunk (16384 slots): expand8 matmul key
    broadcast (PSUM) -> is_equal*cand (VEC) -> bdiag8 matmul reduce
    into ps_s [8, 2048], src+dst accumulated; gate (Ln/sigmoid) reads
    ps_s straight from PSUM. ~14 instructions per chunk, ~340 total.
Host: sorts/groups/pads edges, builds wrapped idx streams, unsorts out.
"""
import sys
sys.path.insert(0, '/opt/trn_rl_repo')
import numpy as np

N, IN_DIM, HID = 50000, 256, 64
E = 1_600_000
BIAS = 0.0001
NCORES = 8
EC = E // NCORES
M = 512
GS = 8                       # slots per src group
NTILES = 53
C = NTILES * M               # 27136 slot-cols/core (x8 groups = slots)
NGROUPS = C                  # src group capacity
TAB_W = 3125                 # N/16
NNC = N // NCORES            # 6250
NNCP = 6272
PC = 2048                    # processing chunk cols
DC = 4096                    # dst gather chunk cols
DCH = (C + DC - 1) // DC     # 7

_nc = None


def _build():
    from concourse import bass, bacc, tile, mybir

    f32 = mybir.dt.float32
    bf16 = mybir.dt.bfloat16
    i16 = mybir.dt.int16
    ACT = mybir.ActivationFunctionType
    OP = mybir.AluOpType
    nc = bacc.Bacc("TRN2", target_bir_lowering=False, debug=False,
                   num_devices=NCORES)

    embT = nc.dram_tensor("embT", [2, 128, NNCP], f32, kind="ExternalInput")
    widx_d = nc.dram_tensor("widx", [128, C // 128], i16,
                            kind="ExternalInput")
    didx_d = nc.dram_tensor("didx", [128, C // 16], i16,
                            kind="ExternalInput")
    skey_d = nc.dram_tensor("skey", [8, C], bf16, kind="ExternalInput")
    dkey_d = nc.dram_tensor("dkey", [8, C], bf16, kind="ExternalInput")
    nz_d = nc.dram_tensor("nz", [8, C], f32, kind="ExternalInput")
    wemb_d = nc.dram_tensor("wemb", [2, 128, HID], f32, kind="ExternalInput")
    bemb_d = nc.dram_tensor("bemb", [HID, 1], f32, kind="ExternalInput")
    wbar_d = nc.dram_tensor("wbar", [HID, 1], f32, kind="ExternalInput")
    bhalf8_d = nc.dram_tensor("bhalf8", [8, 1], f32, kind="ExternalInput")
    expand8_d = nc.dram_tensor("expand8", [8, 128], f32, kind="ExternalInput")
    bdiag8_d = nc.dram_tensor("bdiag8", [128, 8], f32, kind="ExternalInput")
    iota16_d = nc.dram_tensor("iota16", [128, 1], f32, kind="ExternalInput")
    out_d = nc.dram_tensor("out", [8, C], f32, kind="ExternalOutput")

    a1, b1 = 2.0 * BIAS - 1.0, 1.0 - BIAS
    a2, b2 = 1.0 - 2.0 * BIAS, BIAS

    with tile.TileContext(nc) as tc:
        with tc.tile_pool(name="const", bufs=1) as cp, \
             tc.tile_pool(name="tab", bufs=1) as tabp, \
             tc.tile_pool(name="cands", bufs=1) as candsp, \
             tc.tile_pool(name="dram", bufs=1, space="DRAM") as dram:
            def cload(name, shape, dt, src):
                t = cp.tile(shape, dt, tag=name)
                nc.sync.dma_start(out=t[:], in_=src)
                return t
            w0 = cload("w0", [128, HID], f32, wemb_d[0])
            w1 = cload("w1", [128, HID], f32, wemb_d[1])
            bemb = cload("bemb", [HID, 1], f32, bemb_d[:, :])
            wbar = cload("wbar", [HID, 1], f32, wbar_d[:, :])
            bhalf8 = cload("bhalf8", [8, 1], f32, bhalf8_d[:, :])
            expand8_f = cload("ex8f", [8, 128], f32, expand8_d[:, :])
            expand8 = cp.tile([8, 128], bf16, tag="ex8")
            nc.vector.tensor_copy(out=expand8[:], in_=expand8_f[:])
            bdiag8_f = cload("bd8f", [128, 8], f32, bdiag8_d[:, :])
            bdiag8 = cp.tile([128, 8], bf16, tag="bd8")
            nc.vector.tensor_copy(out=bdiag8[:], in_=bdiag8_f[:])
            iota16 = cload("io16", [128, 1], f32, iota16_d[:, :])
            widx = cload("widx", [128, C // 128], i16, widx_d[:, :])
            didx = cload("didx", [128, C // 16], i16, didx_d[:, :])
            a1t = cp.tile([8, 1], f32, tag="a1t"); nc.vector.memset(a1t[:], a1)
            b1t = cp.tile([8, 1], f32, tag="b1t"); nc.vector.memset(b1t[:], b1)
            a2t = cp.tile([8, 1], f32, tag="a2t"); nc.vector.memset(a2t[:], a2)
            b2t = cp.tile([8, 1], f32, tag="b2t"); nc.vector.memset(b2t[:], b2)

            # ---------- phase 1: per-node scalar g ----------
            g_sb = cp.tile([1, NNCP], f32, tag="gsb")
            with tc.tile_pool(name="p1", bufs=3) as p1, \
                 tc.tile_pool(name="ps1", bufs=2, space="PSUM") as ps1, \
                 tc.tile_pool(name="ps1g", bufs=2, space="PSUM") as ps1g:
                col = 0
                while col < NNCP:
                    n = min(512, NNCP - col)
                    r0 = p1.tile([128, n], f32, tag="r0")
                    r1 = p1.tile([128, n], f32, tag="r1")
                    nc.sync.dma_start(out=r0[:], in_=embT[0, :, col:col + n])
                    nc.sync.dma_start(out=r1[:], in_=embT[1, :, col:col + n])
                    ph = ps1.tile([HID, n], f32, tag="ph")
                    nc.tensor.matmul(out=ph[:], lhsT=w0[:], rhs=r0[:],
                                     start=True, stop=False)
                    nc.tensor.matmul(out=ph[:], lhsT=w1[:], rhs=r1[:],
                                     start=False, stop=True)
                    hT = p1.tile([HID, n], f32, tag="hT")
                    nc.scalar.activation(out=hT[:], in_=ph[:], func=ACT.Relu,
                                         bias=bemb[:, 0:1])
                    pg = ps1g.tile([1, n], f32, tag="pg")
                    nc.tensor.matmul(out=pg[:], lhsT=wbar[:], rhs=hT[:],
                                     start=True, stop=True)
                    nc.scalar.activation(out=g_sb[0:1, col:col + n], in_=pg[:],
                                         func=ACT.Identity)
                    col += n

            g_mine = dram.tile([1, NNC], f32, tag="gmine")
            g_all = dram.tile([1, N], f32, tag="gall")
            nc.sync.dma_start(out=g_mine[:], in_=g_sb[0:1, 0:NNC])
            nc.gpsimd.collective_compute(
                "AllGather", bass.mybir.AluOpType.bypass,
                replica_groups=[list(range(NCORES))],
                ins=[g_mine[:].opt()], outs=[g_all[:].opt()])

            table = tabp.tile([128, TAB_W], f32, tag="table")
            g_all_wc = g_all[0].rearrange("(w c) -> c w", c=16)
            for G in range(8):
                nc.sync.dma_start(out=table[16 * G:16 * G + 16, :],
                                  in_=g_all_wc)

            # ---------- phase 2 ----------
            cand_s = candsp.tile([128, C // 8], f32, tag="cands")
            nc.gpsimd.ap_gather(
                out_ap=cand_s[:].rearrange("p (n d) -> p n d", d=1),
                in_ap=table[:].rearrange("p (n d) -> p n d", d=1),
                idxs_ap=widx[:, :], channels=128, num_elems=TAB_W, d=1,
                num_idxs=C // 8)

            with tc.tile_pool(name="candd", bufs=2) as canddp, \
                 tc.tile_pool(name="st", bufs=2) as stp, \
                 tc.tile_pool(name="msk", bufs=3) as mskp, \
                 tc.tile_pool(name="gt", bufs=1) as gtp, \
                 tc.tile_pool(name="psd", bufs=1, space="PSUM") as psd, \
                 tc.tile_pool(name="pss", bufs=1, space="PSUM") as pss:
                cand_d = {}
                for gc in range(DCH):
                    d0 = gc * DC
                    dn = min(DC, C - d0)
                    cd = canddp.tile([128, DC], f32, tag="cd")
                    nc.gpsimd.ap_gather(
                        out_ap=cd[:, 0:dn].rearrange("p (n d) -> p n d", d=1),
                        in_ap=table[:].rearrange("p (n d) -> p n d", d=1),
                        idxs_ap=didx[:, d0 // 16:(d0 + dn) // 16],
                        channels=128, num_elems=TAB_W, d=1, num_idxs=dn)
                    cand_d[gc] = cd

                for c0 in range(0, C, PC):
                    n = min(PC, C - c0)
                    gc, off = c0 // DC, c0 % DC
                    skt = stp.tile([8, n], bf16, tag="skt")
                    nc.sync.dma_start(out=skt[:], in_=skey_d[:, c0:c0 + n])
                    dkt = stp.tile([8, n], bf16, tag="dkt")
                    nc.sync.dma_start(out=dkt[:], in_=dkey_d[:, c0:c0 + n])
                    nzt = stp.tile([8, n], f32, tag="nzt")
                    nc.sync.dma_start(out=nzt[:], in_=nz_d[:, c0:c0 + n])

                    ps_s = pss.tile([8, PC], f32, tag="ps_s")
                    ps_d = psd.tile([128, PC], f32, tag="psd")
                    # src (PE matmuls in 512-col psum-bank-aligned slices)
                    for q0 in range(0, n, 512):
                        qn = min(512, n - q0)
                        nc.tensor.matmul(out=ps_d[:, q0:q0 + qn],
                                         lhsT=expand8[:],
                                         rhs=skt[:, q0:q0 + qn],
                                         start=True, stop=True)
                    cbc = cand_s[:, c0 // 8:(c0 + n) // 8] \
                        .unsqueeze(2).broadcast_to([128, n // 8, GS])
                    msk = mskp.tile([128, n], bf16, tag="msk")
                    nc.vector.scalar_tensor_tensor(
                        out=msk[:], in0=ps_d[:, 0:n], scalar=iota16[:, 0:1],
                        in1=cbc, op0=OP.is_equal, op1=OP.mult)
                    for q0 in range(0, n, 512):
                        qn = min(512, n - q0)
                        nc.tensor.matmul(out=ps_s[:, q0:q0 + qn],
                                         lhsT=bdiag8[:],
                                         rhs=msk[:, q0:q0 + qn],
                                         start=True, stop=True)
                    ssrc = gtp.tile([8, n], f32, tag="ssrc")
                    nc.scalar.activation(out=ssrc[:], in_=ps_s[:, 0:n],
                                         func=ACT.Identity)
                    # dst
                    for q0 in range(0, n, 512):
                        qn = min(512, n - q0)
                        nc.tensor.matmul(out=ps_d[:, q0:q0 + qn],
                                         lhsT=expand8[:],
                                         rhs=dkt[:, q0:q0 + qn],
                                         start=True, stop=True)
                    msk2 = mskp.tile([128, n], bf16, tag="msk2")
                    nc.vector.scalar_tensor_tensor(
                        out=msk2[:], in0=ps_d[:, 0:n], scalar=iota16[:, 0:1],
                        in1=cand_d[gc][:, off:off + n],
                        op0=OP.is_equal, op1=OP.mult)
                    for q0 in range(0, n, 512):
                        qn = min(512, n - q0)
                        nc.tensor.matmul(out=ps_s[:, q0:q0 + qn],
                                         lhsT=bdiag8[:],
                                         rhs=msk2[:, q0:q0 + qn],
                                         start=True, stop=True)
                    # gate
                    t1 = gtp.tile([8, n], f32, tag="t1")
                    nc.scalar.activation(out=t1[:], in_=nzt[:], func=ACT.Ln,
                                         bias=b1t[:, 0:1], scale=a1t[:, 0:1])
                    t2 = gtp.tile([8, n], f32, tag="t2")
                    nc.scalar.activation(out=t2[:], in_=nzt[:], func=ACT.Ln,
                                         bias=b2t[:, 0:1], scale=a2t[:, 0:1])
                    gd = gtp.tile([8, n], f32, tag="gd")
                    nc.vector.scalar_tensor_tensor(
                        out=gd[:], in0=t1[:], scalar=0.0, in1=t2[:],
                        op0=OP.add, op1=OP.subtract)
                    gt2 = gtp.tile([8, n], f32, tag="gt2")
                    nc.vector.scalar_tensor_tensor(
                        out=gt2[:], in0=ps_s[:, 0:n], scalar=bhalf8[:, 0:1],
                        in1=gd[:], op0=OP.add, op1=OP.add)
                    gt3 = gtp.tile([8, n], f32, tag="gt3")
                    nc.vector.tensor_tensor(out=gt3[:], in0=gt2[:],
                                            in1=ssrc[:], op=OP.add)
                    ot = gtp.tile([8, n], f32, tag="ot")
                    nc.scalar.activation(out=ot[:], in_=gt3[:],
                                         func=ACT.Sigmoid)
                    nc.sync.dma_start(out=out_d[:, c0:c0 + n], in_=ot[:])
    nc.compile()
    return nc


def _get_nc():
    global _nc
    if _nc is None:
        _nc = _build()
    return _nc


def _prep_core(e0, e1, nz):
    """Sort by e0, group GS-per-bucket, build device streams.

    Slot linear order (matches out_d [8, C]): G*C + t*M + j*GS + s
    where group index gi = t*512 + G*64 + j.
    """
    EC_ = e0.shape[0]
    ordr = np.argsort(e0, kind='stable')
    e0s, e1s, nzs_ = e0[ordr], e1[ordr], nz[ordr]
    w = e0s >> 4
    bstart = np.searchsorted(w, np.arange(TAB_W), side='left')
    bend = np.searchsorted(w, np.arange(TAB_W), side='right')
    cnt = bend - bstart
    ngr = (cnt + GS - 1) // GS
    tot_groups = int(ngr.sum())
    assert tot_groups <= NGROUPS, (tot_groups, NGROUPS)
    gw = np.repeat(np.arange(TAB_W), ngr).astype(np.int16)
    within = np.concatenate([np.arange(x) for x in ngr])
    gstart = np.repeat(bstart, ngr) + within * GS
    gend = np.minimum(np.repeat(bend, ngr), gstart + GS)

    skey = np.zeros(NGROUPS * GS, dtype=np.float32)
    e1_slot = np.zeros(NGROUPS * GS, dtype=np.int64)
    nz_slot = np.full(NGROUPS * GS, 0.5, dtype=np.float32)
    src_slot = np.full(NGROUPS * GS, -1, dtype=np.int64)
    gw_full = np.zeros(NGROUPS, dtype=np.int16)
    gw_full[:tot_groups] = gw
    lens = (gend - gstart).astype(np.int64)
    gi_rep = np.repeat(np.arange(tot_groups), lens)
    s_rep = np.concatenate([np.arange(x) for x in lens])
    slot_ids = gi_rep * GS + s_rep
    edge_ids = np.repeat(gstart, lens) + s_rep
    skey[slot_ids] = (e0s[edge_ids] & 15).astype(np.float32)
    e1_slot[slot_ids] = e1s[edge_ids]
    nz_slot[slot_ids] = nzs_[edge_ids]
    src_slot[slot_ids] = ordr[edge_ids]

    gi = np.arange(NGROUPS)
    t_of = gi // M
    G_of = (gi % M) // 64
    j_of = gi % 64

    skey_dev = np.zeros((8, C), dtype=np.float32)
    dkey_dev = np.zeros((8, C), dtype=np.float32)
    nz_dev = np.full((8, C), 0.5, dtype=np.float32)
    e1w_dev = np.zeros((8, C), dtype=np.int64)
    e1r = e1_slot.reshape(NGROUPS, GS)
    skr = skey.reshape(NGROUPS, GS)
    nzr = nz_slot.reshape(NGROUPS, GS)
    cols = t_of * M + j_of * GS
    colidx = cols[:, None] + np.arange(GS)[None, :]
    Grep = G_of[:, None].repeat(GS, 1)
    skey_dev[Grep, colidx] = skr
    dkey_dev[Grep, colidx] = (e1r & 15)
    nz_dev[Grep, colidx] = nzr
    e1w_dev[Grep, colidx] = (e1r >> 4)

    dev_slot = np.full(EC_, -1, dtype=np.int64)
    valid = src_slot >= 0
    gi_v = np.arange(NGROUPS * GS)[valid] // GS
    s_v = np.arange(NGROUPS * GS)[valid] % GS
    lin = G_of[gi_v] * C + t_of[gi_v] * M + j_of[gi_v] * GS + s_v
    dev_slot[src_slot[valid]] = lin

    # widx [128, C//128]: src group j of tile t at partition 16G + j%16,
    # col 4t + j//16
    widx = np.zeros((128, C // 128), dtype=np.int16)
    widx[16 * G_of + (j_of % 16), 4 * t_of + (j_of // 16)] = gw_full

    # didx [128, C//16]: dst col m of tile t: group G's idx at partition
    # 16G + m%16, col 32t + m//16
    didx = np.zeros((128, C // 16), dtype=np.int16)
    mm = np.arange(C)
    t_all = mm // M
    m_all = mm % M
    for G in range(8):
        didx[16 * G + (m_all % 16), 32 * t_all + (m_all // 16)] = e1w_dev[G]

    return (widx, didx, skey_dev, dkey_dev, nz_dev, dev_slot)


def prepare_in_maps(embedding, edges, noise, W_emb, b_emb, W_edge, b_edge):
    import ml_dtypes
    embedding = np.asarray(embedding, dtype=np.float32)
    edges = np.asarray(edges)
    noise = np.asarray(noise, dtype=np.float32)
    W_emb = np.asarray(W_emb, dtype=np.float32)
    b_emb = np.asarray(b_emb, dtype=np.float32)
    W_edge = np.asarray(W_edge, dtype=np.float32)
    b_edge = np.float32(b_edge)

    wbar = ((W_edge[:HID] + W_edge[HID:]) * 0.5).astype(np.float32)
    wemb = np.ascontiguousarray(W_emb.reshape(2, 128, HID))
    bemb = np.ascontiguousarray(b_emb.reshape(HID, 1))
    wbarr = np.ascontiguousarray(wbar.reshape(HID, 1))
    bhalf8 = np.full((8, 1), b_edge, dtype=np.float32)
    p = np.arange(128)
    expand8 = (p[None, :] // 16 == np.arange(8)[:, None]).astype(np.float32)
    bdiag8 = (p[:, None] // 16 == np.arange(8)[None, :]).astype(np.float32)
    iota16 = (p % 16).astype(np.float32).reshape(128, 1)

    in_maps = []
    slot_maps = []
    for k in range(NCORES):
        e0 = edges[0, k * EC:(k + 1) * EC].astype(np.int64)
        e1 = edges[1, k * EC:(k + 1) * EC].astype(np.int64)
        nz = noise[k * EC:(k + 1) * EC]
        widx, didx, skey, dkey, nzs, dev_slot = _prep_core(e0, e1, nz)
        slot_maps.append(dev_slot)
        sl = embedding[k * NNC:(k + 1) * NNC]
        embT = np.zeros((IN_DIM, NNCP), dtype=np.float32)
        embT[:, :NNC] = sl.T
        in_maps.append({
            "embT": np.ascontiguousarray(embT.reshape(2, 128, NNCP)),
            "widx": widx, "didx": didx,
            "skey": skey.astype(ml_dtypes.bfloat16),
            "dkey": dkey.astype(ml_dtypes.bfloat16),
            "nz": nzs,
            "wemb": wemb, "bemb": bemb, "wbar": wbarr, "bhalf8": bhalf8,
            "expand8": expand8, "bdiag8": bdiag8, "iota16": iota16,
        })
    return in_maps, slot_maps


def kernel(embedding, edges, noise, W_emb, b_emb, W_edge, b_edge):
    from concourse import bass_utils
    nc = _get_nc()
    in_maps, slot_maps = prepare_in_maps(embedding, edges, noise, W_emb,
                                         b_emb, W_edge, b_edge)
    res = bass_utils.run_bass_kernel_spmd(nc, in_maps,
                                          core_ids=list(range(NCORES)))
    out = np.empty(E, dtype=np.float32)
    for k in range(NCORES):
        flat = res.results[k]["out"].reshape(-1)
        out[k * EC:(k + 1) * EC] = flat[slot_maps[k]]
    return out
